# revision 60
# baseline (speedup 1.0000x reference)
"""Trainium2 Bass kernel for nn_CPPN: 3-layer MLP (4->64->64->3, tanh) over
1M pixels + global min/max normalization, data-parallel over 8 NeuronCores.

Layout strategy (per core, NPIX = 131072 pixels):
  - pixels split into 32 "subsets" of 4096 contiguous pixels; subset s lives
    at partitions 32*(s%4) + 4*(s//4) + i (i = input feature), so layer-1
    runs as K=32 matmuls with zero-padded weights, 4 row-groups concurrent.
  - hidden states keep features on partitions ([64|64] per [128, 1024] tile
    = 4 subsets); tanh runs as full-128-lane ACT ops straight out of 2-bank
    PSUM tiles with the bias fused into the activation.
  - layer-2 packs 4 concurrent 64x64 matmuls via (row, col) tile positions,
    swapping output halves on odd column-halves (undone at host unshard).
  - layer-3 uses a block-diagonal [128, 32] weight emitting two subsets' 3
    channels at partition offsets {0,1,2,16,17,18} of a 32-slot, 4 slots
    per PSUM bank; DVE evacuates with fused bias.
  - ACT is software-pipelined (tanh2 of pair t emitted after tanh1 of
    t+1) with a single shared 3-slot PSUM pool so tanh runs back-to-back.
  - global min/max: per-partition running min/max on DVE, gpsimd
    cross-partition reduce, one tiny AllGather, then per-chunk
    normalize+clip+DMA-out interleaved.
"""

import os
import numpy as np

B, N, NI, H, NO = 4, 262144, 4, 64, 3
NCORES = 8
NPIX_TOT = B * N
NPIX = NPIX_TOT // NCORES      # 131072 pixels per core
NSUB = 16                      # subsets per core (8 x rows each: hi+lo)
SUBPIX = NPIX // NSUB          # 8192 pixels per subset
CW = 512                       # matmul moving-dim chunk width
NCHUNK = SUBPIX // CW          # 16 chunks per subset
NT = NSUB * NCHUNK // 4        # 64 pair-tiles (4 subsets per tile)
OST_F = 2 * NT // 4 * 512      # 16384 staged cols
F32MAX = 3.0e38

_CACHE = {}
LAST_RESULTS = None            # test.py reads exec_time_ns from here


def _build_module(mm_dtype_name="bfloat16"):
    import concourse.bass as bass
    import concourse.tile as tile
    from concourse import bacc, mybir
    from concourse.tile import add_dep_helper

    dt = mybir.dt
    alu = mybir.AluOpType
    act = mybir.ActivationFunctionType
    f32 = dt.float32
    mmdt = getattr(dt, mm_dtype_name)

    nc = bacc.Bacc("TRN2", target_bir_lowering=False, debug=False,
                   num_devices=NCORES)

    x_d = nc.dram_tensor("xcore", [128, SUBPIX], mmdt, kind="ExternalInput").ap()
    w1_d = nc.dram_tensor("w1s", [128, 4 * H], mmdt, kind="ExternalInput").ap()
    w2_d = nc.dram_tensor("w2s", [128, H], mmdt, kind="ExternalInput").ap()
    w3_d = nc.dram_tensor("w3bd", [128, 32], mmdt, kind="ExternalInput").ap()
    b1_d = nc.dram_tensor("b1s", [128, 1], f32, kind="ExternalInput").ap()
    b2_d = nc.dram_tensor("b2s", [128, 1], f32, kind="ExternalInput").ap()
    b3_d = nc.dram_tensor("b3s", [128, 1], f32, kind="ExternalInput").ap()
    vm_d = nc.dram_tensor("validm", [128, 1], f32, kind="ExternalInput").ap()
    nb_d = nc.dram_tensor("negb", [128, 1], f32, kind="ExternalInput").ap()
    out_d = nc.dram_tensor("out", [24, OST_F], f32,
                           kind="ExternalOutput").ap()

    cc_in = nc.dram_tensor("cc_in", [8], f32).ap()
    cc_out = nc.dram_tensor("cc_out", [8 * NCORES], f32,
                            addr_space="Shared").ap()
    cc_win = nc.dram_tensor("cc_win", [8], f32).ap()
    cc_wout = nc.dram_tensor("cc_wout", [8 * NCORES], f32,
                             addr_space="Shared").ap()

    with tile.TileContext(nc) as tc:
        with tc.tile_pool(name="const", bufs=1) as const, \
             tc.tile_pool(name="stage", bufs=1) as stage, \
             tc.tile_pool(name="hid", bufs=2) as hid, \
             tc.tile_pool(name="mm", bufs=1) as mmp, \
             tc.tile_pool(name="pmm", bufs=3, space="PSUM") as pmm, \
             tc.tile_pool(name="ps3", bufs=2, space="PSUM") as ps3:

            xin = const.tile([128, SUBPIX], mmdt, tag="xin")
            w1s = const.tile([128, 4 * H], mmdt, tag="w1s")
            w2s = const.tile([128, H], mmdt, tag="w2s")
            w3bd = const.tile([128, 32], mmdt, tag="w3bd")
            b1s = const.tile([128, 1], f32, tag="b1s")
            b2s = const.tile([128, 1], f32, tag="b2s")
            b3s = const.tile([128, 1], f32, tag="b3s")
            vms = const.tile([128, 1], f32, tag="vms")
            nbs = const.tile([128, 1], f32, tag="nbs")

            nc.sync.dma_start(out=w1s[:], in_=w1_d)
            nc.sync.dma_start(out=xin[:, 0:CW], in_=x_d[:, 0:CW])
            nc.sync.dma_start(out=b1s[:], in_=b1_d)
            nc.sync.dma_start(out=w2s[:], in_=w2_d)
            nc.sync.dma_start(out=w3bd[:], in_=w3_d)
            nc.sync.dma_start(out=b2s[:], in_=b2_d)
            nc.sync.dma_start(out=b3s[:], in_=b3_d)
            nc.sync.dma_start(out=vms[:], in_=vm_d)
            nc.sync.dma_start(out=nbs[:], in_=nb_d)
            nc.sync.dma_start(out=xin[:, CW:], in_=x_d[:, CW:])

            # staged pre-norm output: partition 32w + 16a + 4o,
            # free (u//4)*512 + n; u identifies the L3 emission index.
            ostage = stage.tile([128, OST_F], f32, tag="ostage")

            rmin = mmp.tile([128, 1], f32, tag="rmin")
            rmax = mmp.tile([128, 1], f32, tag="rmax")
            nc.vector.memset(rmin[:], F32MAX)
            nc.vector.memset(rmax[:], -F32MAX)

            # warmup collective: nothing waits on it — runs on TOPSP/ncfw
            # under the main loop to absorb the collective first-call cost
            wseed = nc.sync.dma_start(out=cc_win.rearrange("(p x) -> p x",
                                                           x=1),
                                      in_=vms[0:8, :])
            wcoll = nc.gpsimd.collective_compute(
                "AllGather", alu.bypass,
                replica_groups=[list(range(NCORES))],
                ins=[cc_win], outs=[cc_wout])
            add_dep_helper(wcoll.ins, wseed.ins, reason="seed warm AG")

            def emit_l1(t):
                c = t // 4
                p1 = pmm.tile([128, 2 * CW], f32, tag="pmm",
                              name=f"p1t{t}")
                for v in range(2):
                    for a in range(2):
                        s = 4 * (t % 4) + 2 * v + a
                        g, q = s % 4, s // 4
                        nc.tensor.matmul(
                            out=p1[64 * a: 64 * a + 64, CW * v: CW * v + CW],
                            lhsT=w1s[32 * g: 32 * g + 32, H * q: H * q + H],
                            rhs=xin[32 * g: 32 * g + 32,
                                    c * CW: (c + 1) * CW],
                            start=True, stop=True,
                            tile_position=(32 * g, 64 * a))
                return p1

            def emit_tanh1(t, p1):
                h1 = hid.tile([128, 2 * CW], mmdt, tag="h1")
                nc.scalar.activation(h1[:], p1[:], act.Tanh, bias=b1s[:])
                return h1

            def emit_l2(t, h1):
                p2 = pmm.tile([128, 2 * CW], f32, tag="pmm")
                for v in range(2):
                    for a in range(2):
                        # odd column-half swaps output halves so all four
                        # matmuls pack onto disjoint PE subarray quadrants
                        ao = a ^ (v & 1)
                        nc.tensor.matmul(
                            out=p2[64 * ao: 64 * ao + 64,
                                   CW * v: CW * v + CW],
                            lhsT=w2s[64 * a: 64 * a + 64, :],
                            rhs=h1[64 * a: 64 * a + 64,
                                   CW * v: CW * v + CW],
                            start=True, stop=True,
                            tile_position=(64 * a, 64 * ao))
                return p2

            def emit_tanh2(t, p2):
                h2 = hid.tile([128, 2 * CW], mmdt, tag="h2")
                nc.scalar.activation(h2[:], p2[:], act.Tanh, bias=b2s[:])
                return h2

            ps3_box = [None]

            def emit_l3(t, h2):
                for v in range(2):
                    u = 2 * t + v
                    w = u % 4
                    if w == 0:
                        ps3_box[0] = ps3.tile([128, CW], f32, tag="p3",
                                              name=f"p3t{u}")
                    p3 = ps3_box[0]
                    nc.tensor.matmul(
                        out=p3[32 * w: 32 * w + 32, :],
                        lhsT=w3bd[:],
                        rhs=h2[:, CW * v: CW * v + CW],
                        start=True, stop=True,
                        tile_position=(0, 32 * w))
                    if w == 3:
                        ob = ostage[:, (u // 4) * CW:
                                    (u // 4) * CW + CW]
                        nc.vector.tensor_scalar(ob, p3[:], b3s[:], None,
                                                alu.add)
                        cmin = mmp.tile([128, 1], f32, tag="cmin")
                        cmax = mmp.tile([128, 1], f32, tag="cmax")
                        nc.vector.tensor_reduce(cmin[:], ob,
                                                mybir.AxisListType.X, alu.min)
                        nc.vector.tensor_reduce(cmax[:], ob,
                                                mybir.AxisListType.X, alu.max)
                        nc.vector.tensor_tensor(rmin[:], rmin[:], cmin[:],
                                                alu.min)
                        nc.vector.tensor_tensor(rmax[:], rmax[:], cmax[:],
                                                alu.max)

            # ---- software-pipelined main loop ----
            # PE static order: L1(t+1), L2(t), L3(t-1)  — L1 prefill first
            # ACT static order: tanh1(t), tanh2(t-1)    — back-to-back
            p1s, p2s = {0: emit_l1(0)}, {}
            for t in range(NT + 1):
                if t < NT:
                    h1 = emit_tanh1(t, p1s.pop(t))
                    if t + 1 < NT:
                        p1s[t + 1] = emit_l1(t + 1)
                    p2s[t] = emit_l2(t, h1)
                if t - 1 >= 0:
                    tp = t - 1
                    h2 = emit_tanh2(tp, p2s.pop(tp))
                    emit_l3(tp, h2)

            # ---- global min/max via AllGather ----
            mmt = mmp.tile([128, 2], f32, tag="mmt")
            nc.vector.tensor_scalar(mmt[:, 0:1], rmin[:], -1.0, None, alu.mult)
            nc.vector.tensor_copy(mmt[:, 1:2], rmax[:])
            # mask garbage partitions to -inf so they never win the max
            nc.vector.tensor_scalar(mmt[:], mmt[:], vms[:], nbs[:],
                                    alu.mult, alu.add)
            # compact the 128-partition (-min, max) pairs to 8 partitions
            # via a 32x32 transpose + free-dim reduce, so the DRAM gather
            # is 8 descriptors instead of 256
            scr = mmp.tile([128, 32], f32, tag="scr")
            nc.vector.memset(scr[:], -F32MAX)
            nc.vector.tensor_copy(scr[:, 0:2], mmt[:])
            ttr = mmp.tile([128, 32], f32, tag="ttr")
            nc.vector.transpose(ttr[:], scr[:])
            red = mmp.tile([128, 1], f32, tag="red")
            nc.vector.tensor_reduce(red[:], ttr[:], mybir.AxisListType.X,
                                    alu.max)
            gds = []
            for bb in range(4):
                gds.append(nc.sync.dma_start(
                    out=cc_in[2 * bb: 2 * bb + 2],
                    in_=red[32 * bb: 32 * bb + 2, :]))
            coll = nc.gpsimd.collective_compute(
                "AllGather", alu.bypass,
                replica_groups=[list(range(NCORES))],
                ins=[cc_in], outs=[cc_out])
            for gd in gds:
                add_dep_helper(coll.ins, gd.ins,
                               reason="gather before allgather")
            scb = mmp.tile([128, 8 * NCORES], f32, tag="scb")
            bd = nc.sync.dma_start(out=scb[:],
                                   in_=cc_out.partition_broadcast(128))
            add_dep_helper(bd.ins, coll.ins, reason="bcast after allgather")

            scb_v = scb[:].rearrange("p (k x) -> p k x", x=2)
            nmin = mmp.tile([128, 1], f32, tag="nmin")
            gmax = mmp.tile([128, 1], f32, tag="gmax")
            nc.vector.tensor_reduce(nmin[:], scb_v[:, :, 0],
                                    mybir.AxisListType.X, alu.max)
            nc.vector.tensor_reduce(gmax[:], scb_v[:, :, 1],
                                    mybir.AxisListType.X, alu.max)
            rng = mmp.tile([128, 1], f32, tag="rng")
            nc.vector.tensor_tensor(rng[:], gmax[:], nmin[:], alu.add)
            inv = mmp.tile([128, 1], f32, tag="inv")
            nc.vector.reciprocal(inv[:], rng[:])
            off = mmp.tile([128, 1], f32, tag="off")
            nc.vector.tensor_tensor(off[:], nmin[:], inv[:], alu.mult)

            # ---- normalize + store (min/max make clip a no-op up to 1-ulp
            # rounding, matching the reference's clip) ----
            for hh in range(2):
                fs = (OST_F // 2) * hh
                oc = ostage[:, fs: fs + OST_F // 2]
                nc.vector.tensor_scalar(oc, oc, inv[:], off[:],
                                        alu.mult, alu.add)
                # 3 strided partitions per (w, a), 32KB runs; issue across
                # both HWDGE queues (Sync + Scalar) in parallel
                for w in range(4):
                    for a in range(2):
                        p0 = 32 * w + 16 * a
                        sl = ostage[p0: p0 + 12, fs: fs + OST_F // 2]
                        sl = sl.rearrange("(o r) f -> o r f", o=3)[:, 0, :]
                        eng = nc.sync if (w + a) % 2 == 0 else nc.scalar
                        eng.dma_start(
                            out=out_d[6 * w + 3 * a: 6 * w + 3 * a + 3,
                                      fs: fs + OST_F // 2],
                            in_=sl)
    nc.compile()
    return nc


def _host_inputs(x, W1, b1, W2, b2, W3, b3, mm_np=None):
    """Repack full inputs into per-core in_maps (host-side, not HW-timed)."""
    if mm_np is None:
        import ml_dtypes
        mm = os.environ.get("CPPN_MM_DTYPE", "bfloat16")
        mm_np = ml_dtypes.bfloat16 if mm == "bfloat16" else np.float32
    x = np.asarray(x, np.float32).reshape(NPIX_TOT, NI)
    W1 = np.asarray(W1, np.float32)
    b1 = np.asarray(b1, np.float32)
    W2 = np.asarray(W2, np.float32)
    b2 = np.asarray(b2, np.float32)
    W3 = np.asarray(W3, np.float32)
    b3 = np.asarray(b3, np.float32)

    blk = np.zeros((32, 4 * H), np.float32)
    for q in range(4):
        blk[8 * q: 8 * q + 4, H * q: H * q + H] = W1
        blk[8 * q + 4: 8 * q + 8, H * q: H * q + H] = W1
    w1s = np.tile(blk, (4, 1))

    w2s = np.concatenate([W2, W2], axis=0)
    # layer-3 channels at strided columns 4o (+16 for the a=1 half) so the
    # output rows land on partitions covering 12 distinct DMA port groups
    w3bd = np.zeros((128, 32), np.float32)
    for o in range(NO):
        w3bd[0:64, 4 * o] = W3[:, o]
        w3bd[64:128, 16 + 4 * o] = W3[:, o]

    b1s = np.concatenate([b1, b1])[:, None].astype(np.float32)
    b2s = np.concatenate([b2, b2])[:, None].astype(np.float32)
    b3s = np.zeros((128, 1), np.float32)
    vms = np.zeros((128, 1), np.float32)
    nbs = np.full((128, 1), -1.0e30, np.float32)
    for p in range(128):
        if p % 16 in (0, 4, 8):
            b3s[p, 0] = b3[(p % 16) // 4]
            vms[p, 0] = 1.0
            nbs[p, 0] = 0.0

    in_maps = []
    x_hi = x.astype(mm_np)
    x_lo = (x - x_hi.astype(np.float32)).astype(mm_np)
    for k in range(NCORES):
        sh_hi = x_hi[k * NPIX: (k + 1) * NPIX].reshape(NSUB, SUBPIX, NI)
        sh_lo = x_lo[k * NPIX: (k + 1) * NPIX].reshape(NSUB, SUBPIX, NI)
        xcore = np.empty((128, SUBPIX), mm_np)
        for s in range(NSUB):
            g, q = s % 4, s // 4
            p0 = 32 * g + 8 * q
            xcore[p0: p0 + 4, :] = sh_hi[s].T
            xcore[p0 + 4: p0 + 8, :] = sh_lo[s].T
        in_maps.append({
            "xcore": np.ascontiguousarray(xcore),
            "w1s": w1s.astype(mm_np), "w2s": w2s.astype(mm_np),
            "w3bd": w3bd.astype(mm_np),
            "b1s": b1s, "b2s": b2s, "b3s": b3s,
            "validm": vms, "negb": nbs,
        })
    return in_maps


def _unshard(core_outs):
    """[24, OST_F] per core -> [NO, B, N] full output.

    Row j = 6w + 3a + o; col = (u//4)*512 + n with u = 4*blk + w the L3
    emission index; u = 2t + v; subset s = 4*(t%4) + 2v + (a^v) (the
    layer-2 diagonal packing swaps halves on odd column-halves), chunk
    c = t//4; pixel = s*SUBPIX + c*512 + n.
    """
    out = np.empty((NO, NPIX_TOT), np.float32)
    for k in range(NCORES):
        arr = np.asarray(core_outs[k]).reshape(24, OST_F // 512, 512)
        for j in range(24):
            w, a, o = j // 6, (j % 6) // 3, j % 3
            for blk in range(OST_F // 512):
                u = 4 * blk + w
                t, v = u // 2, u % 2
                s = 4 * (t % 4) + 2 * v + (a ^ v)
                c = t // 4
                base = k * NPIX + s * SUBPIX + c * 512
                out[o, base: base + 512] = arr[j, blk, :]
    return out.reshape(NO, B, N)


def kernel(x, W1, b1, W2, b2, W3, b3):
    global LAST_RESULTS
    from concourse.bass_utils import run_bass_kernel_spmd

    mm = os.environ.get("CPPN_MM_DTYPE", "bfloat16")
    if mm not in _CACHE:
        _CACHE[mm] = _build_module(mm)
    nc = _CACHE[mm]

    in_maps = _host_inputs(x, W1, b1, W2, b2, W3, b3)
    res = run_bass_kernel_spmd(nc, in_maps, list(range(NCORES)))
    LAST_RESULTS = res
    return _unshard([res.results[k]["out"] for k in range(NCORES)])


# revision 62
# speedup vs baseline: 1.0449x; 1.0449x over previous
"""Trainium2 Bass kernel for nn_CPPN: 3-layer MLP (4->64->64->3, tanh) over
1M pixels + global min/max normalization, data-parallel over 8 NeuronCores.

Layout strategy (per core, NPIX = 131072 pixels):
  - pixels split into 32 "subsets" of 4096 contiguous pixels; subset s lives
    at partitions 32*(s%4) + 4*(s//4) + i (i = input feature), so layer-1
    runs as K=32 matmuls with zero-padded weights, 4 row-groups concurrent.
  - hidden states keep features on partitions ([64|64] per [128, 1024] tile
    = 4 subsets); tanh runs as full-128-lane ACT ops straight out of 2-bank
    PSUM tiles with the bias fused into the activation.
  - layer-2 packs 4 concurrent 64x64 matmuls via (row, col) tile positions,
    swapping output halves on odd column-halves (undone at host unshard).
  - layer-3 uses a block-diagonal [128, 32] weight emitting two subsets' 3
    channels at partition offsets {0,1,2,16,17,18} of a 32-slot, 4 slots
    per PSUM bank; DVE evacuates with fused bias.
  - ACT is software-pipelined (tanh2 of pair t emitted after tanh1 of
    t+1) with a single shared 3-slot PSUM pool so tanh runs back-to-back.
  - global min/max: per-partition running min/max on DVE, gpsimd
    cross-partition reduce, one tiny AllGather, then per-chunk
    normalize+clip+DMA-out interleaved.
"""

import os
import numpy as np

B, N, NI, H, NO = 4, 262144, 4, 64, 3
NCORES = 8
NPIX_TOT = B * N
NPIX = NPIX_TOT // NCORES      # 131072 pixels per core
NSUB = 16                      # subsets per core (8 x rows each: hi+lo)
SUBPIX = NPIX // NSUB          # 8192 pixels per subset
CW = 512                       # matmul moving-dim chunk width
NCHUNK = SUBPIX // CW          # 16 chunks per subset
NT = NSUB * NCHUNK // 4        # 64 pair-tiles (4 subsets per tile)
OST_F = 2 * NT // 4 * 512      # 16384 staged cols
F32MAX = 3.0e38

_CACHE = {}
LAST_RESULTS = None            # test.py reads exec_time_ns from here


def _build_module(mm_dtype_name="bfloat16"):
    import concourse.bass as bass
    import concourse.tile as tile
    from concourse import bacc, mybir
    from concourse.tile import add_dep_helper

    dt = mybir.dt
    alu = mybir.AluOpType
    act = mybir.ActivationFunctionType
    f32 = dt.float32
    mmdt = getattr(dt, mm_dtype_name)

    nc = bacc.Bacc("TRN2", target_bir_lowering=False, debug=False,
                   num_devices=NCORES)

    x_d = nc.dram_tensor("xcore", [128, SUBPIX], mmdt, kind="ExternalInput").ap()
    w1_d = nc.dram_tensor("w1s", [128, 4 * H], mmdt, kind="ExternalInput").ap()
    w2_d = nc.dram_tensor("w2s", [128, H], mmdt, kind="ExternalInput").ap()
    w3_d = nc.dram_tensor("w3bd", [128, 32], mmdt, kind="ExternalInput").ap()
    b1_d = nc.dram_tensor("b1s", [128, 1], f32, kind="ExternalInput").ap()
    b2_d = nc.dram_tensor("b2s", [128, 1], f32, kind="ExternalInput").ap()
    b3_d = nc.dram_tensor("b3s", [128, 1], f32, kind="ExternalInput").ap()
    vm_d = nc.dram_tensor("validm", [128, 1], f32, kind="ExternalInput").ap()
    nb_d = nc.dram_tensor("negb", [128, 1], f32, kind="ExternalInput").ap()
    out_d = nc.dram_tensor("out", [24, OST_F], f32,
                           kind="ExternalOutput").ap()

    cc_in = nc.dram_tensor("cc_in", [8], f32).ap()
    cc_out = nc.dram_tensor("cc_out", [8 * NCORES], f32,
                            addr_space="Shared").ap()

    with tile.TileContext(nc) as tc:
        with tc.tile_pool(name="const", bufs=1) as const, \
             tc.tile_pool(name="stage", bufs=1) as stage, \
             tc.tile_pool(name="hid", bufs=2) as hid, \
             tc.tile_pool(name="mm", bufs=1) as mmp, \
             tc.tile_pool(name="pmm", bufs=3, space="PSUM") as pmm, \
             tc.tile_pool(name="ps3", bufs=2, space="PSUM") as ps3:

            xin = const.tile([128, SUBPIX], mmdt, tag="xin")
            w1s = const.tile([128, 4 * H], mmdt, tag="w1s")
            w2s = const.tile([128, H], mmdt, tag="w2s")
            w3bd = const.tile([128, 32], mmdt, tag="w3bd")
            b1s = const.tile([128, 1], f32, tag="b1s")
            b2s = const.tile([128, 1], f32, tag="b2s")
            b3s = const.tile([128, 1], f32, tag="b3s")
            vms = const.tile([128, 1], f32, tag="vms")
            nbs = const.tile([128, 1], f32, tag="nbs")

            nc.sync.dma_start(out=w1s[:], in_=w1_d)
            nc.sync.dma_start(out=xin[:, 0:CW], in_=x_d[:, 0:CW])
            nc.sync.dma_start(out=b1s[:], in_=b1_d)
            nc.sync.dma_start(out=w2s[:], in_=w2_d)
            nc.sync.dma_start(out=w3bd[:], in_=w3_d)
            nc.sync.dma_start(out=b2s[:], in_=b2_d)
            nc.sync.dma_start(out=b3s[:], in_=b3_d)
            nc.sync.dma_start(out=vms[:], in_=vm_d)
            nc.sync.dma_start(out=nbs[:], in_=nb_d)
            nc.sync.dma_start(out=xin[:, CW:], in_=x_d[:, CW:])

            # staged pre-norm output: partition 32w + 16a + 4o,
            # free (u//4)*512 + n; u identifies the L3 emission index.
            ostage = stage.tile([128, OST_F], f32, tag="ostage")

            rmin = mmp.tile([128, 1], f32, tag="rmin")
            rmax = mmp.tile([128, 1], f32, tag="rmax")
            nc.vector.memset(rmin[:], F32MAX)
            nc.vector.memset(rmax[:], -F32MAX)

            def emit_l1(t):
                c = t // 4
                p1 = pmm.tile([128, 2 * CW], f32, tag="pmm",
                              name=f"p1t{t}")
                for v in range(2):
                    for a in range(2):
                        s = 4 * (t % 4) + 2 * v + a
                        g, q = s % 4, s // 4
                        nc.tensor.matmul(
                            out=p1[64 * a: 64 * a + 64, CW * v: CW * v + CW],
                            lhsT=w1s[32 * g: 32 * g + 32, H * q: H * q + H],
                            rhs=xin[32 * g: 32 * g + 32,
                                    c * CW: (c + 1) * CW],
                            start=True, stop=True,
                            tile_position=(32 * g, 64 * a))
                return p1

            def emit_tanh1(t, p1):
                h1 = hid.tile([128, 2 * CW], mmdt, tag="h1")
                nc.scalar.activation(h1[:], p1[:], act.Tanh, bias=b1s[:])
                return h1

            def emit_l2(t, h1):
                p2 = pmm.tile([128, 2 * CW], f32, tag="pmm")
                for v in range(2):
                    for a in range(2):
                        # odd column-half swaps output halves so all four
                        # matmuls pack onto disjoint PE subarray quadrants
                        ao = a ^ (v & 1)
                        nc.tensor.matmul(
                            out=p2[64 * ao: 64 * ao + 64,
                                   CW * v: CW * v + CW],
                            lhsT=w2s[64 * a: 64 * a + 64, :],
                            rhs=h1[64 * a: 64 * a + 64,
                                   CW * v: CW * v + CW],
                            start=True, stop=True,
                            tile_position=(64 * a, 64 * ao))
                return p2

            def emit_tanh2(t, p2):
                h2 = hid.tile([128, 2 * CW], mmdt, tag="h2")
                nc.scalar.activation(h2[:], p2[:], act.Tanh, bias=b2s[:])
                return h2

            ps3_box = [None]

            def emit_l3(t, h2):
                for v in range(2):
                    u = 2 * t + v
                    w = u % 4
                    if w == 0:
                        ps3_box[0] = ps3.tile([128, CW], f32, tag="p3",
                                              name=f"p3t{u}")
                    p3 = ps3_box[0]
                    nc.tensor.matmul(
                        out=p3[32 * w: 32 * w + 32, :],
                        lhsT=w3bd[:],
                        rhs=h2[:, CW * v: CW * v + CW],
                        start=True, stop=True,
                        tile_position=(0, 32 * w))
                    if w == 3:
                        ob = ostage[:, (u // 4) * CW:
                                    (u // 4) * CW + CW]
                        nc.vector.tensor_scalar(ob, p3[:], b3s[:], None,
                                                alu.add)
                        cmin = mmp.tile([128, 1], f32, tag="cmin")
                        cmax = mmp.tile([128, 1], f32, tag="cmax")
                        nc.vector.tensor_reduce(cmin[:], ob,
                                                mybir.AxisListType.X, alu.min)
                        nc.vector.tensor_reduce(cmax[:], ob,
                                                mybir.AxisListType.X, alu.max)
                        nc.vector.tensor_tensor(rmin[:], rmin[:], cmin[:],
                                                alu.min)
                        nc.vector.tensor_tensor(rmax[:], rmax[:], cmax[:],
                                                alu.max)

            # ---- software-pipelined main loop ----
            # PE static order: L1(t+1), L2(t), L3(t-1)  — L1 prefill first
            # ACT static order: tanh1(t), tanh2(t-1)    — back-to-back
            p1s, p2s = {0: emit_l1(0)}, {}
            for t in range(NT + 1):
                if t < NT:
                    h1 = emit_tanh1(t, p1s.pop(t))
                    if t + 1 < NT:
                        p1s[t + 1] = emit_l1(t + 1)
                    p2s[t] = emit_l2(t, h1)
                if t - 1 >= 0:
                    tp = t - 1
                    h2 = emit_tanh2(tp, p2s.pop(tp))
                    emit_l3(tp, h2)

            # ---- global min/max via AllGather ----
            mmt = mmp.tile([128, 2], f32, tag="mmt")
            nc.vector.tensor_scalar(mmt[:, 0:1], rmin[:], -1.0, None, alu.mult)
            nc.vector.tensor_copy(mmt[:, 1:2], rmax[:])
            # mask garbage partitions to -inf so they never win the max
            nc.vector.tensor_scalar(mmt[:], mmt[:], vms[:], nbs[:],
                                    alu.mult, alu.add)
            # compact the 128-partition (-min, max) pairs to 8 partitions
            # via a 32x32 transpose + free-dim reduce, so the DRAM gather
            # is 8 descriptors instead of 256
            scr = mmp.tile([128, 32], f32, tag="scr")
            nc.vector.memset(scr[:], -F32MAX)
            nc.vector.tensor_copy(scr[:, 0:2], mmt[:])
            ttr = mmp.tile([128, 32], f32, tag="ttr")
            nc.vector.transpose(ttr[:], scr[:])
            red = mmp.tile([128, 1], f32, tag="red")
            nc.vector.tensor_reduce(red[:], ttr[:], mybir.AxisListType.X,
                                    alu.max)
            gds = []
            for bb in range(4):
                gds.append(nc.sync.dma_start(
                    out=cc_in[2 * bb: 2 * bb + 2],
                    in_=red[32 * bb: 32 * bb + 2, :]))
            coll = nc.gpsimd.collective_compute(
                "AllGather", alu.bypass,
                replica_groups=[list(range(NCORES))],
                ins=[cc_in], outs=[cc_out])
            for gd in gds:
                add_dep_helper(coll.ins, gd.ins,
                               reason="gather before allgather")
            scb = mmp.tile([128, 8 * NCORES], f32, tag="scb")
            bd = nc.sync.dma_start(out=scb[:],
                                   in_=cc_out.partition_broadcast(128))
            add_dep_helper(bd.ins, coll.ins, reason="bcast after allgather")

            scb_v = scb[:].rearrange("p (k x) -> p k x", x=2)
            nmin = mmp.tile([128, 1], f32, tag="nmin")
            gmax = mmp.tile([128, 1], f32, tag="gmax")
            nc.vector.tensor_reduce(nmin[:], scb_v[:, :, 0],
                                    mybir.AxisListType.X, alu.max)
            nc.vector.tensor_reduce(gmax[:], scb_v[:, :, 1],
                                    mybir.AxisListType.X, alu.max)
            rng = mmp.tile([128, 1], f32, tag="rng")
            nc.vector.tensor_tensor(rng[:], gmax[:], nmin[:], alu.add)
            inv = mmp.tile([128, 1], f32, tag="inv")
            nc.vector.reciprocal(inv[:], rng[:])
            off = mmp.tile([128, 1], f32, tag="off")
            nc.vector.tensor_tensor(off[:], nmin[:], inv[:], alu.mult)

            # ---- normalize + store (min/max make clip a no-op up to 1-ulp
            # rounding, matching the reference's clip) ----
            for hh in range(2):
                fs = (OST_F // 2) * hh
                oc = ostage[:, fs: fs + OST_F // 2]
                nc.vector.tensor_scalar(oc, oc, inv[:], off[:],
                                        alu.mult, alu.add)
                # 3 strided partitions per (w, a), 32KB runs; issue across
                # both HWDGE queues (Sync + Scalar) in parallel
                for w in range(4):
                    for a in range(2):
                        p0 = 32 * w + 16 * a
                        sl = ostage[p0: p0 + 12, fs: fs + OST_F // 2]
                        sl = sl.rearrange("(o r) f -> o r f", o=3)[:, 0, :]
                        eng = nc.sync if (w + a) % 2 == 0 else nc.scalar
                        eng.dma_start(
                            out=out_d[6 * w + 3 * a: 6 * w + 3 * a + 3,
                                      fs: fs + OST_F // 2],
                            in_=sl)
    nc.compile()
    return nc


def _host_inputs(x, W1, b1, W2, b2, W3, b3, mm_np=None):
    """Repack full inputs into per-core in_maps (host-side, not HW-timed)."""
    if mm_np is None:
        import ml_dtypes
        mm = os.environ.get("CPPN_MM_DTYPE", "bfloat16")
        mm_np = ml_dtypes.bfloat16 if mm == "bfloat16" else np.float32
    x = np.asarray(x, np.float32).reshape(NPIX_TOT, NI)
    W1 = np.asarray(W1, np.float32)
    b1 = np.asarray(b1, np.float32)
    W2 = np.asarray(W2, np.float32)
    b2 = np.asarray(b2, np.float32)
    W3 = np.asarray(W3, np.float32)
    b3 = np.asarray(b3, np.float32)

    blk = np.zeros((32, 4 * H), np.float32)
    for q in range(4):
        blk[8 * q: 8 * q + 4, H * q: H * q + H] = W1
        blk[8 * q + 4: 8 * q + 8, H * q: H * q + H] = W1
    w1s = np.tile(blk, (4, 1))

    w2s = np.concatenate([W2, W2], axis=0)
    # layer-3 channels at strided columns 4o (+16 for the a=1 half) so the
    # output rows land on partitions covering 12 distinct DMA port groups
    w3bd = np.zeros((128, 32), np.float32)
    for o in range(NO):
        w3bd[0:64, 4 * o] = W3[:, o]
        w3bd[64:128, 16 + 4 * o] = W3[:, o]

    b1s = np.concatenate([b1, b1])[:, None].astype(np.float32)
    b2s = np.concatenate([b2, b2])[:, None].astype(np.float32)
    b3s = np.zeros((128, 1), np.float32)
    vms = np.zeros((128, 1), np.float32)
    nbs = np.full((128, 1), -1.0e30, np.float32)
    for p in range(128):
        if p % 16 in (0, 4, 8):
            b3s[p, 0] = b3[(p % 16) // 4]
            vms[p, 0] = 1.0
            nbs[p, 0] = 0.0

    in_maps = []
    x_hi = x.astype(mm_np)
    x_lo = (x - x_hi.astype(np.float32)).astype(mm_np)
    for k in range(NCORES):
        sh_hi = x_hi[k * NPIX: (k + 1) * NPIX].reshape(NSUB, SUBPIX, NI)
        sh_lo = x_lo[k * NPIX: (k + 1) * NPIX].reshape(NSUB, SUBPIX, NI)
        xcore = np.empty((128, SUBPIX), mm_np)
        for s in range(NSUB):
            g, q = s % 4, s // 4
            p0 = 32 * g + 8 * q
            xcore[p0: p0 + 4, :] = sh_hi[s].T
            xcore[p0 + 4: p0 + 8, :] = sh_lo[s].T
        in_maps.append({
            "xcore": np.ascontiguousarray(xcore),
            "w1s": w1s.astype(mm_np), "w2s": w2s.astype(mm_np),
            "w3bd": w3bd.astype(mm_np),
            "b1s": b1s, "b2s": b2s, "b3s": b3s,
            "validm": vms, "negb": nbs,
        })
    return in_maps


def _unshard(core_outs):
    """[24, OST_F] per core -> [NO, B, N] full output.

    Row j = 6w + 3a + o; col = (u//4)*512 + n with u = 4*blk + w the L3
    emission index; u = 2t + v; subset s = 4*(t%4) + 2v + (a^v) (the
    layer-2 diagonal packing swaps halves on odd column-halves), chunk
    c = t//4; pixel = s*SUBPIX + c*512 + n.
    """
    out = np.empty((NO, NPIX_TOT), np.float32)
    for k in range(NCORES):
        arr = np.asarray(core_outs[k]).reshape(24, OST_F // 512, 512)
        for j in range(24):
            w, a, o = j // 6, (j % 6) // 3, j % 3
            for blk in range(OST_F // 512):
                u = 4 * blk + w
                t, v = u // 2, u % 2
                s = 4 * (t % 4) + 2 * v + (a ^ v)
                c = t // 4
                base = k * NPIX + s * SUBPIX + c * 512
                out[o, base: base + 512] = arr[j, blk, :]
    return out.reshape(NO, B, N)


def kernel(x, W1, b1, W2, b2, W3, b3):
    global LAST_RESULTS
    from concourse.bass_utils import run_bass_kernel_spmd

    mm = os.environ.get("CPPN_MM_DTYPE", "bfloat16")
    if mm not in _CACHE:
        _CACHE[mm] = _build_module(mm)
    nc = _CACHE[mm]

    in_maps = _host_inputs(x, W1, b1, W2, b2, W3, b3)
    res = run_bass_kernel_spmd(nc, in_maps, list(range(NCORES)))
    LAST_RESULTS = res
    return _unshard([res.results[k]["out"] for k in range(NCORES)])


# revision 66
# speedup vs baseline: 1.0822x; 1.0357x over previous
"""Trainium2 Bass kernel for nn_CPPN: 3-layer MLP (4->64->64->3, tanh) over
1M pixels + global min/max normalization, data-parallel over 8 NeuronCores.

Layout strategy (per core, NPIX = 131072 pixels):
  - pixels split into 32 "subsets" of 4096 contiguous pixels; subset s lives
    at partitions 32*(s%4) + 4*(s//4) + i (i = input feature), so layer-1
    runs as K=32 matmuls with zero-padded weights, 4 row-groups concurrent.
  - hidden states keep features on partitions ([64|64] per [128, 1024] tile
    = 4 subsets); tanh runs as full-128-lane ACT ops straight out of 2-bank
    PSUM tiles with the bias fused into the activation.
  - layer-2 packs 4 concurrent 64x64 matmuls via (row, col) tile positions,
    swapping output halves on odd column-halves (undone at host unshard).
  - layer-3 uses a block-diagonal [128, 32] weight emitting two subsets' 3
    channels at partition offsets {0,1,2,16,17,18} of a 32-slot, 4 slots
    per PSUM bank; DVE evacuates with fused bias.
  - ACT is software-pipelined (tanh2 of pair t emitted after tanh1 of
    t+1) with a single shared 3-slot PSUM pool so tanh runs back-to-back.
  - global min/max: per-partition running min/max on DVE, gpsimd
    cross-partition reduce, one tiny AllGather, then per-chunk
    normalize+clip+DMA-out interleaved.
"""

import os
import numpy as np

B, N, NI, H, NO = 4, 262144, 4, 64, 3
NCORES = 8
NPIX_TOT = B * N
NPIX = NPIX_TOT // NCORES      # 131072 pixels per core
NSUB = 16                      # subsets per core (8 x rows each: hi+lo)
SUBPIX = NPIX // NSUB          # 8192 pixels per subset
CW = 512                       # matmul moving-dim chunk width
NCHUNK = SUBPIX // CW          # 16 chunks per subset
NT = NSUB * NCHUNK // 4        # 64 pair-tiles (4 subsets per tile)
OST_F = 2 * NT // 4 * 512      # 16384 staged cols
F32MAX = 3.0e38

_CACHE = {}
LAST_RESULTS = None            # test.py reads exec_time_ns from here


def _build_module(mm_dtype_name="bfloat16"):
    import concourse.bass as bass
    import concourse.tile as tile
    from concourse import bacc, mybir
    from concourse.tile import add_dep_helper

    dt = mybir.dt
    alu = mybir.AluOpType
    act = mybir.ActivationFunctionType
    f32 = dt.float32
    mmdt = getattr(dt, mm_dtype_name)

    nc = bacc.Bacc("TRN2", target_bir_lowering=False, debug=False,
                   num_devices=NCORES)

    x_d = nc.dram_tensor("xcore", [128, SUBPIX], mmdt, kind="ExternalInput").ap()
    w1_d = nc.dram_tensor("w1s", [128, 4 * H], mmdt, kind="ExternalInput").ap()
    w2_d = nc.dram_tensor("w2s", [128, H], mmdt, kind="ExternalInput").ap()
    w3_d = nc.dram_tensor("w3bd", [128, 32], mmdt, kind="ExternalInput").ap()
    b1_d = nc.dram_tensor("b1s", [128, 1], f32, kind="ExternalInput").ap()
    b2_d = nc.dram_tensor("b2s", [128, 1], f32, kind="ExternalInput").ap()
    b3_d = nc.dram_tensor("b3s", [128, 1], f32, kind="ExternalInput").ap()
    vm_d = nc.dram_tensor("validm", [128, 1], f32, kind="ExternalInput").ap()
    nb_d = nc.dram_tensor("negb", [128, 1], f32, kind="ExternalInput").ap()
    out_d = nc.dram_tensor("out", [24, OST_F], f32,
                           kind="ExternalOutput").ap()

    cc_in = nc.dram_tensor("cc_in", [8], f32).ap()
    cc_out = nc.dram_tensor("cc_out", [8 * NCORES], f32,
                            addr_space="Shared").ap()

    with tile.TileContext(nc) as tc:
        with tc.tile_pool(name="const", bufs=1) as const, \
             tc.tile_pool(name="stage", bufs=1) as stage, \
             tc.tile_pool(name="hid", bufs=2) as hid, \
             tc.tile_pool(name="mm", bufs=1) as mmp, \
             tc.tile_pool(name="pmm", bufs=3, space="PSUM") as pmm, \
             tc.tile_pool(name="ps3", bufs=2, space="PSUM") as ps3:

            xin = const.tile([128, SUBPIX], mmdt, tag="xin")
            w1s = const.tile([128, 4 * H], mmdt, tag="w1s")
            w2s = const.tile([128, H], mmdt, tag="w2s")
            w3bd = const.tile([128, 32], mmdt, tag="w3bd")
            b1s = const.tile([128, 1], f32, tag="b1s")
            b2s = const.tile([128, 1], f32, tag="b2s")
            b3s = const.tile([128, 1], f32, tag="b3s")
            vms = const.tile([128, 1], f32, tag="vms")
            nbs = const.tile([128, 1], f32, tag="nbs")

            nc.sync.dma_start(out=w1s[:], in_=w1_d)
            nc.scalar.dma_start(out=xin[:, 0:CW], in_=x_d[:, 0:CW])
            nc.sync.dma_start(out=b1s[:], in_=b1_d)
            nc.scalar.dma_start(out=w2s[:], in_=w2_d)
            nc.sync.dma_start(out=w3bd[:], in_=w3_d)
            nc.scalar.dma_start(out=b2s[:], in_=b2_d)
            nc.sync.dma_start(out=b3s[:], in_=b3_d)
            nc.scalar.dma_start(out=vms[:], in_=vm_d)
            nc.sync.dma_start(out=nbs[:], in_=nb_d)
            nc.scalar.dma_start(out=xin[:, CW:], in_=x_d[:, CW:])

            # staged pre-norm output: partition 32w + 16a + 4o,
            # free (u//4)*512 + n; u identifies the L3 emission index.
            ostage = stage.tile([128, OST_F], f32, tag="ostage")

            rmin = mmp.tile([128, 1], f32, tag="rmin")
            rmax = mmp.tile([128, 1], f32, tag="rmax")
            nc.vector.memset(rmin[:], F32MAX)
            nc.vector.memset(rmax[:], -F32MAX)

            def emit_l1(t):
                c = t // 4
                p1 = pmm.tile([128, 2 * CW], f32, tag="pmm",
                              name=f"p1t{t}")
                for v in range(2):
                    for a in range(2):
                        s = 4 * (t % 4) + 2 * v + a
                        g, q = s % 4, s // 4
                        nc.tensor.matmul(
                            out=p1[64 * a: 64 * a + 64, CW * v: CW * v + CW],
                            lhsT=w1s[32 * g: 32 * g + 32, H * q: H * q + H],
                            rhs=xin[32 * g: 32 * g + 32,
                                    c * CW: (c + 1) * CW],
                            start=True, stop=True,
                            tile_position=(32 * g, 64 * a))
                return p1

            def emit_tanh1(t, p1):
                h1 = hid.tile([128, 2 * CW], mmdt, tag="h1")
                nc.scalar.activation(h1[:], p1[:], act.Tanh, bias=b1s[:])
                return h1

            def emit_l2(t, h1):
                p2 = pmm.tile([128, 2 * CW], f32, tag="pmm")
                for v in range(2):
                    for a in range(2):
                        # odd column-half swaps output halves so all four
                        # matmuls pack onto disjoint PE subarray quadrants
                        ao = a ^ (v & 1)
                        nc.tensor.matmul(
                            out=p2[64 * ao: 64 * ao + 64,
                                   CW * v: CW * v + CW],
                            lhsT=w2s[64 * a: 64 * a + 64, :],
                            rhs=h1[64 * a: 64 * a + 64,
                                   CW * v: CW * v + CW],
                            start=True, stop=True,
                            tile_position=(64 * a, 64 * ao))
                return p2

            def emit_tanh2(t, p2):
                h2 = hid.tile([128, 2 * CW], mmdt, tag="h2")
                nc.scalar.activation(h2[:], p2[:], act.Tanh, bias=b2s[:])
                return h2

            ps3_box = [None]

            def emit_l3(t, h2):
                for v in range(2):
                    u = 2 * t + v
                    w = u % 4
                    if w == 0:
                        ps3_box[0] = ps3.tile([128, CW], f32, tag="p3",
                                              name=f"p3t{u}")
                    p3 = ps3_box[0]
                    nc.tensor.matmul(
                        out=p3[32 * w: 32 * w + 32, :],
                        lhsT=w3bd[:],
                        rhs=h2[:, CW * v: CW * v + CW],
                        start=True, stop=True,
                        tile_position=(0, 32 * w))
                    if w == 3:
                        ob = ostage[:, (u // 4) * CW:
                                    (u // 4) * CW + CW]
                        nc.vector.tensor_scalar(ob, p3[:], b3s[:], None,
                                                alu.add)
                        cmin = mmp.tile([128, 1], f32, tag="cmin")
                        cmax = mmp.tile([128, 1], f32, tag="cmax")
                        nc.vector.tensor_reduce(cmin[:], ob,
                                                mybir.AxisListType.X, alu.min)
                        nc.vector.tensor_reduce(cmax[:], ob,
                                                mybir.AxisListType.X, alu.max)
                        nc.vector.tensor_tensor(rmin[:], rmin[:], cmin[:],
                                                alu.min)
                        nc.vector.tensor_tensor(rmax[:], rmax[:], cmax[:],
                                                alu.max)

            # ---- software-pipelined main loop ----
            # PE static order: L1(t+1), L2(t), L3(t-1)  — L1 prefill first
            # ACT static order: tanh1(t), tanh2(t-1)    — back-to-back
            p1s, p2s = {0: emit_l1(0)}, {}
            for t in range(NT + 1):
                if t < NT:
                    h1 = emit_tanh1(t, p1s.pop(t))
                    if t + 1 < NT:
                        p1s[t + 1] = emit_l1(t + 1)
                    p2s[t] = emit_l2(t, h1)
                if t - 1 >= 0:
                    tp = t - 1
                    h2 = emit_tanh2(tp, p2s.pop(tp))
                    emit_l3(tp, h2)

            # ---- global min/max via AllGather ----
            mmt = mmp.tile([128, 2], f32, tag="mmt")
            nc.vector.tensor_scalar(mmt[:, 0:1], rmin[:], -1.0, None, alu.mult)
            nc.vector.tensor_copy(mmt[:, 1:2], rmax[:])
            # mask garbage partitions to -inf so they never win the max
            nc.vector.tensor_scalar(mmt[:], mmt[:], vms[:], nbs[:],
                                    alu.mult, alu.add)
            # compact the 128-partition (-min, max) pairs to 8 partitions
            # via a 32x32 transpose + free-dim reduce, so the DRAM gather
            # is 8 descriptors instead of 256
            scr = mmp.tile([128, 32], f32, tag="scr")
            nc.vector.memset(scr[:], -F32MAX)
            nc.vector.tensor_copy(scr[:, 0:2], mmt[:])
            ttr = mmp.tile([128, 32], f32, tag="ttr")
            nc.vector.transpose(ttr[:], scr[:])
            red = mmp.tile([128, 1], f32, tag="red")
            nc.vector.tensor_reduce(red[:], ttr[:], mybir.AxisListType.X,
                                    alu.max)
            gds = []
            for bb in range(4):
                gds.append((nc.sync if bb % 2 == 0 else nc.scalar).dma_start(
                    out=cc_in[2 * bb: 2 * bb + 2],
                    in_=red[32 * bb: 32 * bb + 2, :]))
            coll = nc.gpsimd.collective_compute(
                "AllGather", alu.bypass,
                replica_groups=[list(range(NCORES))],
                ins=[cc_in], outs=[cc_out])
            for gd in gds:
                add_dep_helper(coll.ins, gd.ins,
                               reason="gather before allgather")
            scb = mmp.tile([128, 8 * NCORES], f32, tag="scb")
            bd = nc.sync.dma_start(out=scb[:],
                                   in_=cc_out.partition_broadcast(128))
            add_dep_helper(bd.ins, coll.ins, reason="bcast after allgather")

            scb_v = scb[:].rearrange("p (k x) -> p k x", x=2)
            nmin = mmp.tile([128, 1], f32, tag="nmin")
            gmax = mmp.tile([128, 1], f32, tag="gmax")
            nc.vector.tensor_reduce(nmin[:], scb_v[:, :, 0],
                                    mybir.AxisListType.X, alu.max)
            nc.vector.tensor_reduce(gmax[:], scb_v[:, :, 1],
                                    mybir.AxisListType.X, alu.max)
            rng = mmp.tile([128, 1], f32, tag="rng")
            nc.vector.tensor_tensor(rng[:], gmax[:], nmin[:], alu.add)
            inv = mmp.tile([128, 1], f32, tag="inv")
            nc.vector.reciprocal(inv[:], rng[:])
            off = mmp.tile([128, 1], f32, tag="off")
            nc.vector.tensor_tensor(off[:], nmin[:], inv[:], alu.mult)

            # ---- normalize + store (min/max make clip a no-op up to 1-ulp
            # rounding, matching the reference's clip) ----
            for hh in range(2):
                fs = (OST_F // 2) * hh
                oc = ostage[:, fs: fs + OST_F // 2]
                nc.vector.tensor_scalar(oc, oc, inv[:], off[:],
                                        alu.mult, alu.add)
                # 3 strided partitions per (w, a), 32KB runs; issue across
                # both HWDGE queues (Sync + Scalar) in parallel
                for w in range(4):
                    for a in range(2):
                        p0 = 32 * w + 16 * a
                        sl = ostage[p0: p0 + 12, fs: fs + OST_F // 2]
                        sl = sl.rearrange("(o r) f -> o r f", o=3)[:, 0, :]
                        eng = nc.sync if (w + a) % 2 == 0 else nc.scalar
                        eng.dma_start(
                            out=out_d[6 * w + 3 * a: 6 * w + 3 * a + 3,
                                      fs: fs + OST_F // 2],
                            in_=sl)
    nc.compile()
    return nc


def _host_inputs(x, W1, b1, W2, b2, W3, b3, mm_np=None):
    """Repack full inputs into per-core in_maps (host-side, not HW-timed)."""
    if mm_np is None:
        import ml_dtypes
        mm = os.environ.get("CPPN_MM_DTYPE", "bfloat16")
        mm_np = ml_dtypes.bfloat16 if mm == "bfloat16" else np.float32
    x = np.asarray(x, np.float32).reshape(NPIX_TOT, NI)
    W1 = np.asarray(W1, np.float32)
    b1 = np.asarray(b1, np.float32)
    W2 = np.asarray(W2, np.float32)
    b2 = np.asarray(b2, np.float32)
    W3 = np.asarray(W3, np.float32)
    b3 = np.asarray(b3, np.float32)

    blk = np.zeros((32, 4 * H), np.float32)
    for q in range(4):
        blk[8 * q: 8 * q + 4, H * q: H * q + H] = W1
        blk[8 * q + 4: 8 * q + 8, H * q: H * q + H] = W1
    w1s = np.tile(blk, (4, 1))

    w2s = np.concatenate([W2, W2], axis=0)
    # layer-3 channels at strided columns 4o (+16 for the a=1 half) so the
    # output rows land on partitions covering 12 distinct DMA port groups
    w3bd = np.zeros((128, 32), np.float32)
    for o in range(NO):
        w3bd[0:64, 4 * o] = W3[:, o]
        w3bd[64:128, 16 + 4 * o] = W3[:, o]

    b1s = np.concatenate([b1, b1])[:, None].astype(np.float32)
    b2s = np.concatenate([b2, b2])[:, None].astype(np.float32)
    b3s = np.zeros((128, 1), np.float32)
    vms = np.zeros((128, 1), np.float32)
    nbs = np.full((128, 1), -1.0e30, np.float32)
    for p in range(128):
        if p % 16 in (0, 4, 8):
            b3s[p, 0] = b3[(p % 16) // 4]
            vms[p, 0] = 1.0
            nbs[p, 0] = 0.0

    in_maps = []
    x_hi = x.astype(mm_np)
    x_lo = (x - x_hi.astype(np.float32)).astype(mm_np)
    for k in range(NCORES):
        sh_hi = x_hi[k * NPIX: (k + 1) * NPIX].reshape(NSUB, SUBPIX, NI)
        sh_lo = x_lo[k * NPIX: (k + 1) * NPIX].reshape(NSUB, SUBPIX, NI)
        xcore = np.empty((128, SUBPIX), mm_np)
        for s in range(NSUB):
            g, q = s % 4, s // 4
            p0 = 32 * g + 8 * q
            xcore[p0: p0 + 4, :] = sh_hi[s].T
            xcore[p0 + 4: p0 + 8, :] = sh_lo[s].T
        in_maps.append({
            "xcore": np.ascontiguousarray(xcore),
            "w1s": w1s.astype(mm_np), "w2s": w2s.astype(mm_np),
            "w3bd": w3bd.astype(mm_np),
            "b1s": b1s, "b2s": b2s, "b3s": b3s,
            "validm": vms, "negb": nbs,
        })
    return in_maps


def _unshard(core_outs):
    """[24, OST_F] per core -> [NO, B, N] full output.

    Row j = 6w + 3a + o; col = (u//4)*512 + n with u = 4*blk + w the L3
    emission index; u = 2t + v; subset s = 4*(t%4) + 2v + (a^v) (the
    layer-2 diagonal packing swaps halves on odd column-halves), chunk
    c = t//4; pixel = s*SUBPIX + c*512 + n.
    """
    out = np.empty((NO, NPIX_TOT), np.float32)
    for k in range(NCORES):
        arr = np.asarray(core_outs[k]).reshape(24, OST_F // 512, 512)
        for j in range(24):
            w, a, o = j // 6, (j % 6) // 3, j % 3
            for blk in range(OST_F // 512):
                u = 4 * blk + w
                t, v = u // 2, u % 2
                s = 4 * (t % 4) + 2 * v + (a ^ v)
                c = t // 4
                base = k * NPIX + s * SUBPIX + c * 512
                out[o, base: base + 512] = arr[j, blk, :]
    return out.reshape(NO, B, N)


def kernel(x, W1, b1, W2, b2, W3, b3):
    global LAST_RESULTS
    from concourse.bass_utils import run_bass_kernel_spmd

    mm = os.environ.get("CPPN_MM_DTYPE", "bfloat16")
    if mm not in _CACHE:
        _CACHE[mm] = _build_module(mm)
    nc = _CACHE[mm]

    in_maps = _host_inputs(x, W1, b1, W2, b2, W3, b3)
    res = run_bass_kernel_spmd(nc, in_maps, list(range(NCORES)))
    LAST_RESULTS = res
    return _unshard([res.results[k]["out"] for k in range(NCORES)])
